# revision 24
# baseline (speedup 1.0000x reference)
"""MixtureRouter Trainium2 kernel (v2: w1-stationary, host-normalized fp8).

Per-core (data-parallel over batch, 8 cores): the device computes
    G[r, c] = sum_{t in chunk c} gelu( (xn @ w1g)[t, r] + vb1[r] )
for r-chunks of 128 partitions x token-chunks of 512, i.e. the full
Linear(2048->512) + bias + GELU + sum-over-sequence. The host computes
LayerNorm in f32 (exactly matching reference semantics), folds ln_gamma
into w1, pre-transposes x to [d, tok] fp8 layout, and runs the tiny tail
(H @ w2 + S*b2 -> router head, aux_loss / next_idx) in fp64.

Key design points vs the previous version:
  - Orientation flipped: w1g chunks are the matmul STATIONARY operand
    ([d,r] needs no transpose), xn^T the MOVING operand. The host ships
    x already d-major, so the 256 on-chip PE transposes and the 16 big
    PSUM->SBUF ACT copies are gone entirely. PE issues nothing but the
    128 DoubleRow fp8 matmuls (~213ns each => ~27us, the fp8 roofline).
  - With r on partitions, the b1 bias is a per-partition scalar: it
    rides the ACT Gelu as the `bias` operand, and the token-sum rides
    the same instruction as `accum_out` (free-dim reduction). One ACT
    instruction per PSUM bank does bias+gelu+reduce; DVE is idle.
  - LayerNorm is folded on the host: xn = (x - mu) * rsqrt(var + eps)
    in f32 (better than the device bn_stats path), then quantized to
    fp8e4m3. fp8 x fp8 DoubleRow was already the baseline's matmul
    precision; measured end-to-end logits error ~5e-3 vs the 2e-2 gate,
    and per-batch top-1 margins are ~14+ logits vs ~0.5 error.
  - fp8 x halves HBM traffic again (4 MiB/core + 1 MiB weights), DMAs
    use 512B-contiguous descriptors, spread over the SP and DVE queues,
    and stream tc0 (first token chunk) + stationaries first so the PE
    starts ~2us in. A dummy Gelu on the bias tile pre-loads the ACT
    table during the DMA warmup window.
"""

import sys
import types

import ml_dtypes
import numpy as np

import concourse.bass as bass
import concourse.mybir as mybir
import concourse.tile as tile
from concourse import bacc
from concourse.bass_utils import run_bass_kernel_spmd

# run_bass_kernel_spmd imports antenv.axon_hooks when BASS_TRACE is set; that
# module is absent on this image. Provide it so tracing degrades gracefully.
if "antenv.axon_hooks" not in sys.modules:
    try:
        import antenv.axon_hooks  # noqa: F401
    except ImportError:
        _hm = types.ModuleType("antenv.axon_hooks")
        _hm._hook = None
        _hm.set_axon_ntff_profile_hook = lambda h: setattr(_hm, "_hook", h)
        _hm.get_axon_ntff_profile_hook = lambda: _hm._hook
        sys.modules["antenv.axon_hooks"] = _hm
        try:
            from trn_agent_boot.trn_boot import _ntff_profile_via_ctypes

            _hm._hook = _ntff_profile_via_ctypes("/opt/axon/libaxon_pjrt.so")
        except Exception:
            pass

F32 = mybir.dt.float32
FP8 = mybir.dt.float8e4

B, S, D, R, E = 8, 2048, 2048, 512, 8
N_CORES = 8
P = 128
NK = D // P          # 16 contraction chunks of 128
NK2 = NK // 2        # 8 DoubleRow pairs
NTC = S // 512       # 4 token chunks of 512
NRC = R // P         # 4 r chunks of 128
LN_EPS = 1e-5

_cache = {}


def _build():
    nc = bacc.Bacc("TRN2", target_bir_lowering=False, debug=False, num_devices=N_CORES)
    # xd rows: ((tc*8 + k2)*128 + p), cols: (j*512 + t)  [fp8, 4 MiB]
    xd = nc.dram_tensor("xd", [NTC * NK2 * P, 2 * 512], FP8, kind="ExternalInput")
    # wd rows: (a*128 + p), cols: (jj*512 + r), a in 0..3 [fp8, 1 MiB]
    wd = nc.dram_tensor("wd", [4 * P, 4 * R], FP8, kind="ExternalInput")
    vd = nc.dram_tensor("vd", [P, NRC], F32, kind="ExternalInput")
    gout = nc.dram_tensor("gout", [P, NTC * NRC], F32, kind="ExternalOutput")

    with tile.TileContext(nc) as tc:
        with (
            tc.tile_pool(name="const", bufs=1) as const,
            tc.tile_pool(name="gdump", bufs=2) as gdump,
            tc.tile_pool(name="psm", bufs=8, space="PSUM") as psm,
        ):
            ws = const.tile([P, NK, R], FP8)      # stationary w1g  (8 KiB/part)
            # moving xn^T, tc-major so every x DMA writes 1024B-contiguous
            # per-partition runs (row tci*NK + k holds tokens of chunk tci)
            xs = const.tile([P, NTC * NK, 512], FP8)   # 32 KiB/part
            vb1s = const.tile([P, NRC], F32)      # bias b1 (gamma/beta folded)
            gcol = const.tile([P, NTC * NRC], F32)
            scr1 = const.tile([P, 1], F32)
            # PE warm-up junk operands (zeroed; real matmul shapes)
            wj = const.tile([P, 2, P], FP8)
            xj = const.tile([P, 2, 512], FP8)

            def w_slice(a):
                return wd[a * P : (a + 1) * P, :].rearrange(
                    "p (jj r) -> p jj r", r=R
                )

            def x_slice(tci, k2):
                base = (tci * NK2 + k2) * P
                return xd[base : base + P, :].rearrange("p (j t) -> p j t", t=512)

            def x_dst(tci, k2):
                row = tci * NK + 2 * k2
                return xs[:, row : row + 2, :]

            def x_mov(tci, k2):
                row = tci * NK + 2 * k2
                return xs[:, row : row + 2, :]

            nc.vector.memset(scr1, 0.0)
            nc.vector.memset(wj.bitcast(mybir.dt.uint32), 0)
            nc.vector.memset(xj.bitcast(mybir.dt.uint32), 0)

            # prime the ACT Gelu table (no data deps: immediate scale/bias,
            # zeroed input); vd rides the ACT HWDGE queue (tiny transfer)
            nc.scalar.activation(
                scr1, scr1, mybir.ActivationFunctionType.Gelu, bias=0.0, scale=0.0
            )
            nc.scalar.dma_start(vb1s, vd[:, :])

            # DMA schedule: bulk split across the two fast queues (the ACT
            # HWDGE queue is ~10x slower, bulk never goes there). The tc0
            # phase is DMA-paced (w + x = 2 MiB), so items are interleaved in
            # PE demand order across the queues: SP carries the w quads
            # (2048B descriptors) + tc0-odd, SWDGE carries tc0-even; after
            # the head, evens go to SP and odds to SWDGE.
            def w_dma(q, a):
                q.dma_start(ws[:, 4 * a : 4 * a + 4], w_slice(a))

            def x_dma(q, tci, k2):
                q.dma_start(x_dst(tci, k2), x_slice(tci, k2))

            for a in range(4):
                w_dma(nc.sync, a)
                x_dma(nc.gpsimd, 0, 2 * a)
                x_dma(nc.sync, 0, 2 * a + 1)
            for tci in (1, 2, 3):
                for k2 in range(NK2):
                    x_dma(nc.sync if k2 % 2 == 0 else nc.gpsimd, tci, k2)

            # compute. tc0..tc2: k2-outer over 4 concurrent PSUM banks, so the
            # first matmul only needs one 128 KiB x chunk and PE consumption
            # paces DMA delivery. tc3: rc-outer so the tail GELUs overlap the
            # last matmuls instead of serializing after them.
            for tci in range(NTC):
                banks = [
                    psm.tile([P, 512], F32, tag="mm", name=f"mm{tci}_{rc}")
                    for rc in range(NRC)
                ]

                if tci == 0:
                    # burn the PE DVFS ramp on junk matmuls (results are
                    # discarded; bank is reset by the first real start=True
                    # matmul). The first 6 have no data deps; the last 3 use
                    # the real w0 quad as stationary so the PE stays busy
                    # until the first x chunk lands — an idle gap here would
                    # re-throttle the clock.
                    for i in range(6):
                        nc.tensor.matmul(
                            banks[0], wj, xj, start=True, stop=True,
                            perf_mode=mybir.MatmulPerfMode.DoubleRow,
                            skip_group_check=True,
                        )
                    for i in range(3):
                        nc.tensor.matmul(
                            banks[0], ws[:, 0:2, 0:P], xj, start=True,
                            stop=True,
                            perf_mode=mybir.MatmulPerfMode.DoubleRow,
                            skip_group_check=True,
                        )

                def mm(rc, k2):
                    nc.tensor.matmul(
                        banks[rc],
                        ws[:, 2 * k2 : 2 * k2 + 2, rc * P : (rc + 1) * P],
                        x_mov(tci, k2),
                        start=(k2 == 0), stop=(k2 == NK2 - 1),
                        perf_mode=mybir.MatmulPerfMode.DoubleRow,
                        skip_group_check=True,
                    )

                def gelu(rc):
                    g = gdump.tile([P, 512], F32, tag="g", name=f"g{tci}_{rc}")
                    col = tci * NRC + rc
                    nc.scalar.activation(
                        g, banks[rc], mybir.ActivationFunctionType.Gelu,
                        bias=vb1s[:, rc : rc + 1],
                        accum_out=gcol[:, col : col + 1],
                    )

                if tci < NTC - 1:
                    for k2 in range(NK2):
                        for rc in range(NRC):
                            mm(rc, k2)
                    for rc in range(NRC):
                        gelu(rc)
                    # ship this token chunk's 4 columns while compute continues
                    nc.scalar.dma_start(
                        gout[:, tci * NRC : (tci + 1) * NRC],
                        gcol[:, tci * NRC : (tci + 1) * NRC],
                    )
                else:
                    # tail: per-rc gelu + 2 KiB gout so the final DMA waits
                    # only on the last bank's gelu
                    for rc in range(NRC):
                        for k2 in range(NK2):
                            mm(rc, k2)
                        gelu(rc)
                        col = tci * NRC + rc
                        nc.scalar.dma_start(
                            gout[:, col : col + 1], gcol[:, col : col + 1]
                        )
    nc.finalize()
    return nc


def kernel(hidden_states, ln_gamma, ln_beta, w1, b1, w2, b2, wr, br):
    hs = np.asarray(hidden_states, dtype=np.float32)
    # LayerNorm on host in f32 (f64 accumulation for the stats)
    mu = hs.mean(-1, keepdims=True, dtype=np.float64)
    var = (hs.astype(np.float64) - mu).var(-1, keepdims=True)
    rstd = 1.0 / np.sqrt(var + LN_EPS)
    xn8 = ((hs - mu.astype(np.float32)) * rstd.astype(np.float32)).astype(
        ml_dtypes.float8_e4m3fn
    )

    g64 = np.asarray(ln_gamma, dtype=np.float64)
    be64 = np.asarray(ln_beta, dtype=np.float64)
    w1_64 = np.asarray(w1, dtype=np.float64)
    w1g8 = (g64[:, None] * w1_64).astype(np.float32).astype(ml_dtypes.float8_e4m3fn)
    vb1 = (be64 @ w1_64 + np.asarray(b1, np.float64)).astype(np.float32)

    # device layouts (see _build):
    #   wd[(a*128+p), (jj*512+r)] = w1g8[(4*a+jj)*128+p, r]
    wdh = np.ascontiguousarray(
        w1g8.reshape(4, 4, P, R).transpose(0, 2, 1, 3).reshape(4 * P, 4 * R)
    )
    vdh = np.ascontiguousarray(vb1.reshape(NRC, P).T)  # [128, 4]

    if "nc" not in _cache:
        _cache["nc"] = _build()
    nc = _cache["nc"]

    in_maps = []
    for b in range(N_CORES):
        #   xd[((tc*8+k2)*128+p), (j*512+t)] = xn8[b, tc*512+t, (2*k2+j)*128+p]
        xT = np.ascontiguousarray(xn8[b].T)  # [D, S]
        xdh = np.ascontiguousarray(
            xT.reshape(NK2, 2, P, NTC, 512)
            .transpose(3, 0, 2, 1, 4)
            .reshape(NTC * NK2 * P, 2 * 512)
        )
        in_maps.append({"xd": xdh, "wd": wdh, "vd": vdh})
    res = run_bass_kernel_spmd(nc, in_maps, core_ids=list(range(N_CORES)))
    gaccs = np.stack([res.results[b]["gout"] for b in range(N_CORES)], axis=0)
    global _last_res
    _last_res = res

    # host tail in fp64 (tiny): H -> w2 -> router -> aux/next_idx
    # gcol[p, tc*4+rc] = sum over token chunk tc of gelu row r = rc*128+p
    H = (
        gaccs.astype(np.float64)
        .reshape(B, P, NTC, NRC)
        .sum(axis=2)            # [B, p, rc]
        .transpose(0, 2, 1)     # [B, rc, p]
        .reshape(B, R)
    )
    bt = H @ np.asarray(w2, np.float64) + float(S) * np.asarray(b2, np.float64)
    logits = bt @ np.asarray(wr, np.float64) + np.asarray(br, np.float64)  # [B, E]
    global _last_logits
    _last_logits = logits.astype(np.float32)

    idx = logits.argmax(axis=-1)
    targets = np.zeros_like(logits)
    targets[np.arange(B), idx] = 1.0
    aux = (np.logaddexp(0.0, logits) - logits * targets).mean()
    counts = targets.sum(0)
    next_idx = int(np.argmax(counts))
    return np.float32(aux), np.int32(next_idx)


# revision 30
# speedup vs baseline: 1.0229x; 1.0229x over previous
"""MixtureRouter Trainium2 kernel (v2: w1-stationary, host-normalized fp8).

Per-core (data-parallel over batch, 8 cores): the device computes
    G[r, c] = sum_{t in chunk c} gelu( (xn @ w1g)[t, r] + vb1[r] )
for r-chunks of 128 partitions x token-chunks of 512, i.e. the full
Linear(2048->512) + bias + GELU + sum-over-sequence. The host computes
LayerNorm in f32 (exactly matching reference semantics), folds ln_gamma
into w1, pre-transposes x to [d, tok] fp8 layout, and runs the tiny tail
(H @ w2 + S*b2 -> router head, aux_loss / next_idx) in fp64.

Key design points vs the previous version:
  - Orientation flipped: w1g chunks are the matmul STATIONARY operand
    ([d,r] needs no transpose), xn^T the MOVING operand. The host ships
    x already d-major, so the 256 on-chip PE transposes and the 16 big
    PSUM->SBUF ACT copies are gone entirely. PE issues nothing but the
    128 DoubleRow fp8 matmuls (~213ns each => ~27us, the fp8 roofline).
  - With r on partitions, the b1 bias is a per-partition scalar: it
    rides the ACT Gelu as the `bias` operand, and the token-sum rides
    the same instruction as `accum_out` (free-dim reduction). One ACT
    instruction per PSUM bank does bias+gelu+reduce; DVE is idle.
  - LayerNorm is folded on the host: xn = (x - mu) * rsqrt(var + eps)
    in f32 (better than the device bn_stats path), then quantized to
    fp8e4m3. fp8 x fp8 DoubleRow was already the baseline's matmul
    precision; measured end-to-end logits error ~5e-3 vs the 2e-2 gate,
    and per-batch top-1 margins are ~14+ logits vs ~0.5 error.
  - fp8 x halves HBM traffic again (4 MiB/core + 1 MiB weights), DMAs
    use 512B-contiguous descriptors, spread over the SP and DVE queues,
    and stream tc0 (first token chunk) + stationaries first so the PE
    starts ~2us in. A dummy Gelu on the bias tile pre-loads the ACT
    table during the DMA warmup window.
"""

import sys
import types

import ml_dtypes
import numpy as np

import concourse.bass as bass
import concourse.mybir as mybir
import concourse.tile as tile
from concourse import bacc
from concourse.bass_utils import run_bass_kernel_spmd

# run_bass_kernel_spmd imports antenv.axon_hooks when BASS_TRACE is set; that
# module is absent on this image. Provide it so tracing degrades gracefully.
if "antenv.axon_hooks" not in sys.modules:
    try:
        import antenv.axon_hooks  # noqa: F401
    except ImportError:
        _hm = types.ModuleType("antenv.axon_hooks")
        _hm._hook = None
        _hm.set_axon_ntff_profile_hook = lambda h: setattr(_hm, "_hook", h)
        _hm.get_axon_ntff_profile_hook = lambda: _hm._hook
        sys.modules["antenv.axon_hooks"] = _hm
        try:
            from trn_agent_boot.trn_boot import _ntff_profile_via_ctypes

            _hm._hook = _ntff_profile_via_ctypes("/opt/axon/libaxon_pjrt.so")
        except Exception:
            pass

F32 = mybir.dt.float32
FP8 = mybir.dt.float8e4

B, S, D, R, E = 8, 2048, 2048, 512, 8
N_CORES = 8
P = 128
NK = D // P          # 16 contraction chunks of 128
NK2 = NK // 2        # 8 DoubleRow pairs
NTC = S // 512       # 4 token chunks of 512
NRC = R // P         # 4 r chunks of 128
LN_EPS = 1e-5

_cache = {}


def _build():
    nc = bacc.Bacc("TRN2", target_bir_lowering=False, debug=False, num_devices=N_CORES)
    # xd rows: ((u*16 + k)*128 + p), cols: t in token half u  [fp8, 4 MiB]
    xd = nc.dram_tensor("xd", [2 * NK * P, 1024], FP8, kind="ExternalInput")
    # wd rows: (a*128 + p), cols: (jj*512 + r), a in 0..3 [fp8, 1 MiB]
    wd = nc.dram_tensor("wd", [4 * P, 4 * R], FP8, kind="ExternalInput")
    vd = nc.dram_tensor("vd", [P, NRC], F32, kind="ExternalInput")
    gout = nc.dram_tensor("gout", [P, NTC * NRC], F32, kind="ExternalOutput")

    with tile.TileContext(nc) as tc:
        with (
            tc.tile_pool(name="const", bufs=1) as const,
            tc.tile_pool(name="gdump", bufs=2) as gdump,
            tc.tile_pool(name="psm", bufs=8, space="PSUM") as psm,
        ):
            ws = const.tile([P, NK, R], FP8)      # stationary w1g  (8 KiB/part)
            # moving xn^T: k-major rows of 2048 tokens. The 2048B row stride
            # matters: DoubleRow reads both k-tile rows concurrently, and
            # 512B-strided rows collide in SBUF (measured 216 -> 259 ns).
            xs = const.tile([P, NK, S], FP8)      # 32 KiB/part
            vb1s = const.tile([P, NRC], F32)      # bias b1 (gamma/beta folded)
            gcol = const.tile([P, NTC * NRC], F32)
            scr1 = const.tile([P, 1], F32)
            # PE warm-up junk operands (zeroed; real matmul shapes)
            wj = const.tile([P, 2, P], FP8)
            xj = const.tile([P, 2, 512], FP8)

            def w_slice(a):
                return wd[a * P : (a + 1) * P, :].rearrange(
                    "p (jj r) -> p jj r", r=R
                )

            def x_mov(tci, k2):
                return xs[:, 2 * k2 : 2 * k2 + 2, tci * 512 : (tci + 1) * 512]

            nc.vector.memset(scr1, 0.0)
            nc.vector.memset(wj.bitcast(mybir.dt.uint32), 0)
            nc.vector.memset(xj.bitcast(mybir.dt.uint32), 0)

            # prime the ACT Gelu table (no data deps: immediate scale/bias,
            # zeroed input); vd rides the ACT HWDGE queue (tiny transfer)
            nc.scalar.activation(
                scr1, scr1, mybir.ActivationFunctionType.Gelu, bias=0.0, scale=0.0
            )
            nc.scalar.dma_start(vb1s, vd[:, :])

            # DMA schedule: bulk split across the two fast queues (the ACT
            # HWDGE queue is ~10x slower, bulk never goes there). x moves as
            # 32 per-(token-half u, k-chunk) DMAs of 128 KiB with 1024B
            # descriptors (~165 B/ns on SP); w as 4 quads with 2048B
            # descriptors. SWDGE is trigger-paced (~0.8us per DMA regardless
            # of size). The u0 half + w feed both tc0 and tc1 and are
            # emitted strictly in PE demand order, alternating queues.
            def w_dma(q, a):
                q.dma_start(ws[:, 4 * a : 4 * a + 4], w_slice(a))

            def x_dma(q, u, k):
                base = (u * NK + k) * P
                q.dma_start(
                    xs[:, k, u * 1024 : (u + 1) * 1024], xd[base : base + P, :]
                )

            for u in range(2):
                items = []
                for a in range(4):
                    items.append(("w", a))
                    items.extend(("x", 4 * a + i) for i in range(4))
                for idx, (kind, val) in enumerate(items):
                    q = nc.sync if idx % 2 == 0 else nc.gpsimd
                    if kind == "w":
                        if u == 0:
                            w_dma(q, val)
                        # u == 1: weights already resident; keep x alternating
                        else:
                            continue
                    else:
                        x_dma(q, u, val)

            # compute. tc0..tc2: k2-outer over 4 concurrent PSUM banks, so the
            # first matmul only needs one 128 KiB x chunk and PE consumption
            # paces DMA delivery. tc3: rc-outer so the tail GELUs overlap the
            # last matmuls instead of serializing after them.
            for tci in range(NTC):
                banks = [
                    psm.tile([P, 512], F32, tag="mm", name=f"mm{tci}_{rc}")
                    for rc in range(NRC)
                ]

                if tci == 0:
                    # burn the PE DVFS ramp on junk matmuls (results are
                    # discarded; bank is reset by the first real start=True
                    # matmul). The first 6 have no data deps; the last 3 use
                    # the real w0 quad as stationary so the PE stays busy
                    # until the first x chunk lands — an idle gap here would
                    # re-throttle the clock.
                    for i in range(6):
                        nc.tensor.matmul(
                            banks[0], wj, xj, start=True, stop=True,
                            perf_mode=mybir.MatmulPerfMode.DoubleRow,
                            skip_group_check=True,
                        )
                    for i in range(3):
                        nc.tensor.matmul(
                            banks[0], ws[:, 0:2, 0:P], xj, start=True,
                            stop=True,
                            perf_mode=mybir.MatmulPerfMode.DoubleRow,
                            skip_group_check=True,
                        )

                def mm(rc, k2):
                    nc.tensor.matmul(
                        banks[rc],
                        ws[:, 2 * k2 : 2 * k2 + 2, rc * P : (rc + 1) * P],
                        x_mov(tci, k2),
                        start=(k2 == 0), stop=(k2 == NK2 - 1),
                        perf_mode=mybir.MatmulPerfMode.DoubleRow,
                        skip_group_check=True,
                    )

                def gelu(rc):
                    g = gdump.tile([P, 512], F32, tag="g", name=f"g{tci}_{rc}")
                    col = tci * NRC + rc
                    nc.scalar.activation(
                        g, banks[rc], mybir.ActivationFunctionType.Gelu,
                        bias=vb1s[:, rc : rc + 1],
                        accum_out=gcol[:, col : col + 1],
                    )

                if tci < NTC - 1:
                    for k2 in range(NK2):
                        for rc in range(NRC):
                            mm(rc, k2)
                    for rc in range(NRC):
                        gelu(rc)
                    # ship this token chunk's 4 columns while compute continues
                    nc.scalar.dma_start(
                        gout[:, tci * NRC : (tci + 1) * NRC],
                        gcol[:, tci * NRC : (tci + 1) * NRC],
                    )
                else:
                    # tail: rc-outer so each bank's gelu overlaps the next
                    # bank's matmuls; one 2 KiB gout after the last gelu
                    for rc in range(NRC):
                        for k2 in range(NK2):
                            mm(rc, k2)
                        gelu(rc)
                    nc.scalar.dma_start(
                        gout[:, tci * NRC : (tci + 1) * NRC],
                        gcol[:, tci * NRC : (tci + 1) * NRC],
                    )
    nc.finalize()
    return nc


def kernel(hidden_states, ln_gamma, ln_beta, w1, b1, w2, b2, wr, br):
    hs = np.asarray(hidden_states, dtype=np.float32)
    # LayerNorm on host in f32 (f64 accumulation for the stats)
    mu = hs.mean(-1, keepdims=True, dtype=np.float64)
    var = (hs.astype(np.float64) - mu).var(-1, keepdims=True)
    rstd = 1.0 / np.sqrt(var + LN_EPS)
    xn8 = ((hs - mu.astype(np.float32)) * rstd.astype(np.float32)).astype(
        ml_dtypes.float8_e4m3fn
    )

    g64 = np.asarray(ln_gamma, dtype=np.float64)
    be64 = np.asarray(ln_beta, dtype=np.float64)
    w1_64 = np.asarray(w1, dtype=np.float64)
    w1g8 = (g64[:, None] * w1_64).astype(np.float32).astype(ml_dtypes.float8_e4m3fn)
    vb1 = (be64 @ w1_64 + np.asarray(b1, np.float64)).astype(np.float32)

    # device layouts (see _build):
    #   wd[(a*128+p), (jj*512+r)] = w1g8[(4*a+jj)*128+p, r]
    wdh = np.ascontiguousarray(
        w1g8.reshape(4, 4, P, R).transpose(0, 2, 1, 3).reshape(4 * P, 4 * R)
    )
    vdh = np.ascontiguousarray(vb1.reshape(NRC, P).T)  # [128, 4]

    if "nc" not in _cache:
        _cache["nc"] = _build()
    nc = _cache["nc"]

    in_maps = []
    for b in range(N_CORES):
        #   xd[((u*16+k)*128+p), t] = xn8[b, u*1024+t, k*128+p]
        xT = np.ascontiguousarray(xn8[b].T)  # [D, S]
        xdh = np.ascontiguousarray(
            xT.reshape(NK, P, 2, 1024)
            .transpose(2, 0, 1, 3)
            .reshape(2 * NK * P, 1024)
        )
        in_maps.append({"xd": xdh, "wd": wdh, "vd": vdh})
    res = run_bass_kernel_spmd(nc, in_maps, core_ids=list(range(N_CORES)))
    gaccs = np.stack([res.results[b]["gout"] for b in range(N_CORES)], axis=0)
    global _last_res
    _last_res = res

    # host tail in fp64 (tiny): H -> w2 -> router -> aux/next_idx
    # gcol[p, tc*4+rc] = sum over token chunk tc of gelu row r = rc*128+p
    H = (
        gaccs.astype(np.float64)
        .reshape(B, P, NTC, NRC)
        .sum(axis=2)            # [B, p, rc]
        .transpose(0, 2, 1)     # [B, rc, p]
        .reshape(B, R)
    )
    bt = H @ np.asarray(w2, np.float64) + float(S) * np.asarray(b2, np.float64)
    logits = bt @ np.asarray(wr, np.float64) + np.asarray(br, np.float64)  # [B, E]
    global _last_logits
    _last_logits = logits.astype(np.float32)

    idx = logits.argmax(axis=-1)
    targets = np.zeros_like(logits)
    targets[np.arange(B), idx] = 1.0
    aux = (np.logaddexp(0.0, logits) - logits * targets).mean()
    counts = targets.sum(0)
    next_idx = int(np.argmax(counts))
    return np.float32(aux), np.int32(next_idx)
